# revision 29
# baseline (speedup 1.0000x reference)
"""Trainium2 Bass kernel for a dense transformer block (B=2, T=2048, C=1024, 16 heads).

Sharding: data-parallel over batch (2 groups of 4 cores) x tensor-parallel
within each group (4 heads + 1024 MLP hidden per core). The T=2048 rows are
processed as four 512-row chunks through a software-pipelined schedule:

  LN1+QKV(chunk) -> attention(chunk) -> out-proj -> AllReduce(bf16)
    -> residual+LN2 (replicated in group) -> MLP -> ReduceScatter(bf16) -> out

v2 changes vs v1 (trace-driven):
  - LN1 for chunks 1-3 is sequence-parallel: each core normalizes+transposes
    only its own 128 rows per chunk; one merged AllGather distributes hT.
  - LN rstd computed as exp(-0.5*ln(var+eps)) so the whole kernel (attention
    exp included) stays in the natural_log_exp ACT table set: no table thrash.
  - Attention softmax epilogue: instead of transposing A^T to normalize rows,
    the denominator row is partition-broadcast on GpSimd and A^T columns are
    divided in place; out-proj consumes the normalized A^T directly via
    contract-64 matmuls (kills the PE<->DVE transpose ping-pong).
  - Scores for both 64-channel subheads share one [128,2,512] PSUM tile and
    a single Exp instruction.
  - MLP gelu reads PSUM directly with the bias fused (no DVE copy stage).
  - b_o/TP folded into the out-proj PSUM evacuation; chunk-3 MLP row-split
    so the tail pipelines LN2-tile -> MLP1-tile.
"""
import functools
import os
import sys
import types

sys.path.insert(0, "/opt/trn_rl_repo")

import numpy as np
import ml_dtypes

import concourse.bass as bass
import concourse.mybir as mybir
from concourse import tile
import concourse.bass_utils as bass_utils

BF16 = ml_dtypes.bfloat16
F32 = np.float32
dt = mybir.dt
AF = mybir.ActivationFunctionType
ALU = mybir.AluOpType

B, T, C = 2, 2048, 1024
NH, HS = 16, 64
NCORES = 8
TP = 4                      # tensor-parallel group size
GROUPS = [[0, 1, 2, 3], [4, 5, 6, 7]]
HPR = NH // TP              # heads per rank
CHR = HPR * HS              # attn channels per rank (256)
HIDR = 4 * C // TP          # MLP hidden per rank (1024)
RPC = T // TP               # rows per core (512)
EPS = 1e-5
NCT = C // 128              # C tiles (8)
NRT = T // 128              # row tiles over full T (16)


# ---------------------------------------------------------------------------
# Harness fixups: the walrus in this container caps sync-wait commands per
# instruction, but Tile's kernel-tail drain carries one wait per active
# processor. Split those waits onto individual SP nops ahead of the drain.
def _patched_drain_and_barrier(self, tick_clock, wait_clock):
    nc = self.nc
    probe = mybir.InstNoOp(
        name=nc.get_next_instruction_name(),
        engine=mybir.EngineType.SP,
        bass_nofuse=True,
    )
    wait_clock.add_sem_waits(probe, tile.ScopedClock({None: tick_clock.global_clock}))
    waits = list(probe.sync_info.on_wait) if probe.sync_info is not None else []
    for w in waits:
        nop = nc.sync.nop(nofuse=True, hint="split_tail_wait")
        nop.ins.sync_info = mybir.SyncInfo(on_wait=[w], on_update=[])
    nc.sync.drain()
    nc.all_engine_barrier()
    assert self.sems is not None
    popped = nc._tile_sem_poison_stack.pop()
    assert popped is self._sem_poison
    nc.clear_and_free_semaphores(list(self.sems.allocated().values()))
    nc.all_engine_barrier()


tile.TileContext._drain_and_barrier = _patched_drain_and_barrier


def _install_ntff_hook():
    """antenv.axon_hooks is absent from this image; provide it and register
    the ctypes NTFF profile hook so trace=True yields exec_time_ns."""
    if "antenv.axon_hooks" in sys.modules:
        return
    import antenv

    mod = types.ModuleType("antenv.axon_hooks")
    mod._hook = None
    mod.set_axon_ntff_profile_hook = lambda h: setattr(mod, "_hook", h)
    mod.get_axon_ntff_profile_hook = lambda: mod._hook
    sys.modules["antenv.axon_hooks"] = mod
    antenv.axon_hooks = mod
    try:
        from trn_agent_boot.trn_boot import _ntff_profile_via_ctypes

        hook = _ntff_profile_via_ctypes("/opt/axon/libaxon_pjrt.so")
        if hook is not None:
            mod.set_axon_ntff_profile_hook(hook)
    except Exception:
        pass
    bass_utils.upload_artifacts = lambda tmpdir: f"local://{tmpdir}"

    import concourse.bass2jax as b2j

    orig_hook = b2j.neuronx_cc_hook

    def dbg_hook(*a, **k):
        try:
            return orig_hook(*a, **k)
        except BaseException:
            import traceback

            traceback.print_exc()
            raise

    b2j.neuronx_cc_hook = dbg_hook


_install_ntff_hook()


_SYNC_WAIT_LIMIT = 1


def _split_sync_waits(nc, limit=_SYNC_WAIT_LIMIT):
    """Walrus in this container rejects instructions with more than a couple
    of sync-wait commands; hoist excess waits onto same-engine NOPs placed
    immediately before the offending instruction."""
    n_split = 0
    for fn in nc.m.functions:
        for bb in fn.blocks:
            new_insts = []
            for inst in bb.instructions:
                si = inst.sync_info
                if si is not None and si.on_wait is not None and len(si.on_wait) > limit:
                    waits = list(si.on_wait)
                    for idx, w in enumerate(waits[limit:]):
                        nop = mybir.InstNoOp(
                            name=f"{inst.name}-sw{idx}",
                            engine=inst.engine,
                            bass_nofuse=True,
                            sync_info=mybir.SyncInfo(on_wait=[w], on_update=[]),
                        )
                        new_insts.append(nop)
                        n_split += 1
                    inst.sync_info = mybir.SyncInfo(
                        on_wait=waits[:limit], on_update=list(si.on_update)
                    )
                new_insts.append(inst)
            bb.instructions = new_insts
    return n_split


# ---------------------------------------------------------------------------
def _build_nc() -> bass.Bass:
    nc = bass.Bass("TRN2", num_devices=NCORES, num_swdge_queues=4)

    x_b = nc.dram_tensor("x_b", [T, C], dt.float32, kind="ExternalInput")
    x_own = nc.dram_tensor("x_own", [3 * 128, C], dt.float32, kind="ExternalInput")
    wq = nc.dram_tensor("wq", [C, CHR], dt.bfloat16, kind="ExternalInput")
    wk = nc.dram_tensor("wk", [C, CHR], dt.bfloat16, kind="ExternalInput")
    wv = nc.dram_tensor("wv", [C, CHR], dt.bfloat16, kind="ExternalInput")
    bq = nc.dram_tensor("bq", [128, 2], dt.float32, kind="ExternalInput")
    bk = nc.dram_tensor("bk", [128, 2], dt.float32, kind="ExternalInput")
    bvb = nc.dram_tensor("bvb", [128, CHR], dt.float32, kind="ExternalInput")
    wo = nc.dram_tensor("wo", [CHR, C], dt.bfloat16, kind="ExternalInput")
    bob = nc.dram_tensor("bob", [128, C], dt.float32, kind="ExternalInput")
    w1 = nc.dram_tensor("w1", [C, HIDR], dt.bfloat16, kind="ExternalInput")
    b1 = nc.dram_tensor("b1", [128, HIDR // 128], dt.float32, kind="ExternalInput")
    w2 = nc.dram_tensor("w2", [HIDR, C], dt.bfloat16, kind="ExternalInput")
    bq4 = nc.dram_tensor("bq4", [128, C], dt.float32, kind="ExternalInput")
    ident = nc.dram_tensor("ident", [128, 128], dt.bfloat16, kind="ExternalInput")
    maskut = nc.dram_tensor("maskut", [128, 128], dt.bfloat16, kind="ExternalInput")
    out = nc.dram_tensor("out", [RPC, C], dt.bfloat16, kind="ExternalOutput")

    with tile.TileContext(nc) as tc:
        with (
            tc.tile_pool(name="dram", bufs=1, space="DRAM") as dram,
            tc.tile_pool(name="const", bufs=1) as cpool,
            tc.tile_pool(name="kqv", bufs=1) as kqvpool,
        ):
            rs1_in = [dram.tile([512, C], dt.bfloat16, name=f"rs1i{rc}", tag=f"rs1i{rc}") for rc in range(3)]
            ar1_out = [dram.tile([512, C], dt.bfloat16, name=f"ar1o{rc}", tag=f"ar1o{rc}") for rc in range(3)]
            rs1h = [dram.tile([256, C], dt.bfloat16, name=f"rs1h{k}", tag=f"rs1h{k}") for k in range(2)]
            ar1h = [dram.tile([256, C], dt.bfloat16, name=f"ar1h{k}", tag=f"ar1h{k}") for k in range(2)]
            rs2_in = [dram.tile([512, C], dt.bfloat16, name=f"rs2i{rc}", tag=f"rs2i{rc}") for rc in range(TP)]
            rs2_out = [dram.tile([128, C], dt.bfloat16, name=f"rs2o{rc}", tag=f"rs2o{rc}") for rc in range(TP)]
            ag1i = [dram.tile([NCT, 128, 128], dt.bfloat16, name=f"ag1i{k}", tag=f"ag1i{k}") for k in range(3)]
            ag1o = [dram.tile([TP, NCT, 128, 128], dt.bfloat16, name=f"ag1o{k}", tag=f"ag1o{k}") for k in range(3)]
            warm_in = dram.tile([128, 4], dt.float32, name="warm_i", tag="warm_i")
            warm_out = dram.tile([TP * 128, 4], dt.float32, name="warm_o", tag="warm_o")
            nc.gpsimd.collective_compute(
                "AllGather", ALU.bypass, replica_groups=GROUPS,
                ins=[warm_in[:].opt()], outs=[warm_out[:].opt()],
            )

            # ---- weights/constants to SBUF
            wq_sb = cpool.tile([128, NCT, CHR], dt.bfloat16, name="wq", tag="wq")
            wk_sb = cpool.tile([128, NCT, CHR], dt.bfloat16, name="wk", tag="wk")
            wv_sb = cpool.tile([128, NCT, CHR], dt.bfloat16, name="wv", tag="wv")
            nc.scalar.dma_start(wq_sb[:], wq.rearrange("(j p) o -> p j o", p=128))
            nc.scalar.dma_start(wk_sb[:], wk.rearrange("(j p) o -> p j o", p=128))
            nc.scalar.dma_start(wv_sb[:], wv.rearrange("(j p) o -> p j o", p=128))
            wo_sb = cpool.tile([128, 2, C], dt.bfloat16, name="wo", tag="wo")
            nc.scalar.dma_start(wo_sb[:], wo.rearrange("(t p) o -> p t o", p=128))
            w1_sb = cpool.tile([128, NCT, HIDR], dt.bfloat16, name="w1", tag="w1")
            nc.scalar.dma_start(w1_sb[:], w1.rearrange("(j p) o -> p j o", p=128))
            w2_sb = cpool.tile([128, HIDR // 128, C], dt.bfloat16, name="w2", tag="w2")
            nc.scalar.dma_start(w2_sb[:], w2.rearrange("(j p) o -> p j o", p=128))
            bq_sb = cpool.tile([128, 2], dt.float32, name="bq", tag="bq")
            bk_sb = cpool.tile([128, 2], dt.float32, name="bk", tag="bk")
            nc.scalar.dma_start(bq_sb[:], bq[:])
            nc.scalar.dma_start(bk_sb[:], bk[:])
            bvb_sb = cpool.tile([128, CHR], dt.float32, name="bvb", tag="bvb")
            nc.scalar.dma_start(bvb_sb[:], bvb[:])
            bob_sb = cpool.tile([128, C], dt.float32, name="bob", tag="bob")
            nc.scalar.dma_start(bob_sb[:], bob[:])
            b1_sb = cpool.tile([128, HIDR // 128], dt.float32, name="b1", tag="b1")
            nc.scalar.dma_start(b1_sb[:], b1[:])
            bq4_sb = cpool.tile([128, C], dt.float32, name="bq4", tag="bq4")
            nc.scalar.dma_start(bq4_sb[:], bq4[:])
            id_sb = cpool.tile([128, 128], dt.bfloat16, name="id", tag="id")
            nc.sync.dma_start(id_sb[:], ident[:])
            mask_sb = cpool.tile([128, 128], dt.bfloat16, name="mask", tag="mask")
            nc.sync.dma_start(mask_sb[:], maskut[:])
            eps_sb = cpool.tile([128, 1], dt.float32, name="eps", tag="eps")
            nc.vector.memset(eps_sb[:], EPS)
            ones1_sb = cpool.tile([1, 64], dt.bfloat16, name="ones1", tag="ones1")
            nc.vector.memset(ones1_sb[:], 1.0)

            kt = [kqvpool.tile([128, T], dt.bfloat16, name=f"kt{h2}", tag=f"kt{h2}") for h2 in range(2)]
            vaug = kqvpool.tile([128, NRT, HPR, HS + 1], dt.bfloat16, name="vaug", tag="vaug")
            h2T = [kqvpool.tile([128, NCT, 512], dt.bfloat16, name=f"h2T{rc}", tag=f"h2T{rc % 2}") for rc in range(TP)]
            zb_tiles = [[None] * 4 for _ in range(TP)]
            hT_tiles = [None] * TP
            qt_tiles = [None] * TP
            avn_tiles = [[None] * 2 for _ in range(TP)]

            from contextlib import ExitStack
            with ExitStack() as stack:
                ep = stack.enter_context
                lxpool = ep(tc.tile_pool(name="lnx", bufs=2))
                spool = ep(tc.tile_pool(name="lnsp", bufs=6))
                hpool = ep(tc.tile_pool(name="lnh", bufs=3))
                hTpool = ep(tc.tile_pool(name="hTc", bufs=2))
                htopool = ep(tc.tile_pool(name="hTo", bufs=2))
                qtpool = ep(tc.tile_pool(name="qtc", bufs=2))
                zpool = ep(tc.tile_pool(name="zt", bufs=2))
                zbpool = ep(tc.tile_pool(name="zb", bufs=6))
                ptpool = ep(tc.tile_pool(name="pt", bufs=3))
                anpool = ep(tc.tile_pool(name="anat", bufs=2))
                dbpool = ep(tc.tile_pool(name="dnb", bufs=2))
                avnpool = ep(tc.tile_pool(name="avn", bufs=2))
                obpool = ep(tc.tile_pool(name="ob", bufs=2))
                gtpool = ep(tc.tile_pool(name="gt", bufs=8))
                utpool = ep(tc.tile_pool(name="ut", bufs=8))
                g3pool = ep(tc.tile_pool(name="g3", bufs=1))
                mbpool = ep(tc.tile_pool(name="mb", bufs=2))
                psb = ep(tc.tile_pool(name="psb", bufs=3, space="PSUM"))
                ps2pool = ep(tc.tile_pool(name="ps2", bufs=1, space="PSUM"))
                psapool = ep(tc.tile_pool(name="psa", bufs=2, space="PSUM"))
                # LN of a [128, C] f32 tile -> bf16 (gain/bias folded downstream)
                # rstd = exp(-0.5*ln(var+eps)): keeps ACT in natural_log_exp set.
                def ln_tile(src_ap, dst_ap):
                    st6 = spool.tile([128, 2, 6], dt.float32, name="st6", tag="st6")
                    nc.vector.bn_stats(st6[:, 0, :], src_ap[:, 0:512])
                    nc.vector.bn_stats(st6[:, 1, :], src_ap[:, 512:1024])
                    st2 = spool.tile([128, 2], dt.float32, name="st2", tag="st2")
                    nc.vector.bn_aggr(st2[:], st6[:])
                    lnv = spool.tile([128, 1], dt.float32, name="lnv", tag="lnv")
                    nc.scalar.activation(lnv[:], st2[:, 1:2], AF.Ln, bias=eps_sb[:])
                    rstd = spool.tile([128, 1], dt.float32, name="rstd", tag="rstd")
                    nc.scalar.activation(rstd[:], lnv[:], AF.Exp, scale=-0.5)
                    nc.vector.tensor_scalar(
                        dst_ap, src_ap, st2[:, 0:1], rstd[:],
                        op0=ALU.subtract, op1=ALU.mult,
                    )

                def transpose_128(dst_ap, src_ap, vec=False):
                    pst = psb.tile([128, 128], dt.bfloat16, name="pst", tag="psb")
                    nc.tensor.transpose(pst[:], src_ap, id_sb[:])
                    if vec:
                        nc.vector.tensor_copy(dst_ap, pst[:])
                    else:
                        nc.scalar.copy(dst_ap, pst[:])

                def ln1_own():
                    """Sequence-parallel LN1 for chunks 1-3 (own 128 rows each);
                    per-chunk AllGather of the transposed slices so chunk 1's
                    hT is available as early as possible."""
                    for idx in range(3):
                        xt = lxpool.tile([128, C], dt.float32, name="xt", tag="xt")
                        nc.sync.dma_start(xt[:], x_own[idx * 128:(idx + 1) * 128, :])
                        ho = hpool.tile([128, C], dt.bfloat16, name="h", tag="h")
                        ln_tile(xt[:], ho[:])
                        hTo = htopool.tile([128, NCT, 128], dt.bfloat16, name="hTo", tag="hTo")
                        for j in range(NCT):
                            transpose_128(hTo[:, j, :], ho[:, j * 128:(j + 1) * 128])
                        nc.sync.dma_start(ag1i[idx][:].rearrange("j p c -> p j c"), hTo[:])
                        nc.gpsimd.collective_compute(
                            "AllGather", ALU.bypass, replica_groups=GROUPS,
                            ins=[ag1i[idx][:].opt()], outs=[ag1o[idx][:].opt()],
                        )
                        yield

                def lnqkv_steps(cc):
                    """hT for chunk cc (local LN for cc=0, AG DMA-in for cc>0),
                    then K/Q/V projections for the chunk (generator)."""
                    hT = hTpool.tile([128, NCT, 512], dt.bfloat16, name="hTc", tag="hTc")
                    hT_tiles[cc] = hT
                    if cc == 0:
                        for tl in range(4):
                            xt = lxpool.tile([128, C], dt.float32, name="xt", tag="xt")
                            nc.sync.dma_start(xt[:], x_b[tl * 128:(tl + 1) * 128, :])
                            h = hpool.tile([128, C], dt.bfloat16, name="h", tag="h")
                            ln_tile(xt[:], h[:])
                            for j in range(NCT):
                                transpose_128(hT[:, j, tl * 128:(tl + 1) * 128],
                                              h[:, j * 128:(j + 1) * 128])
                            yield
                    else:
                        for r in range(TP):
                            nc.sync.dma_start(
                                hT[:, :, r * 128:(r + 1) * 128],
                                ag1o[cc - 1][r].rearrange("j p c -> p j c"),
                            )
                        yield
                    qt = qtpool.tile([128, 2, 512], dt.bfloat16, name="qtc", tag="qtc")
                    qt_tiles[cc] = qt
                    for h2 in range(2):
                        for w_sb, is_q, b_sb in ((wk_sb, False, bk_sb), (wq_sb, True, bq_sb)):
                            ps = psb.tile([128, 512], dt.float32, name="psqk", tag="psb")
                            for j in range(NCT):
                                nc.tensor.matmul(
                                    ps[:],
                                    w_sb[:, j, h2 * 128:(h2 + 1) * 128],
                                    hT[:, j, :],
                                    start=(j == 0), stop=(j == NCT - 1),
                                )
                            if is_q:
                                nc.scalar.activation(
                                    qt[:, h2, :], ps[:],
                                    AF.Identity, bias=b_sb[:, h2:h2 + 1],
                                )
                            else:
                                nc.scalar.activation(
                                    kt[h2][:, cc * 512:(cc + 1) * 512], ps[:],
                                    AF.Identity, bias=b_sb[:, h2:h2 + 1],
                                )
                            yield
                    for tl in range(4):
                        i = cc * 4 + tl
                        ps = psb.tile([128, CHR], dt.float32, name="psv", tag="psb")
                        for j in range(NCT):
                            nc.tensor.matmul(
                                ps[:],
                                hT[:, j, tl * 128:(tl + 1) * 128],
                                wv_sb[:, j, :],
                                start=(j == 0), stop=(j == NCT - 1),
                            )
                        nc.vector.tensor_tensor(
                            vaug[:, i, :, 0:HS],
                            ps[:].rearrange("p (h d) -> p h d", d=HS),
                            bvb_sb[:].rearrange("p (h d) -> p h d", d=HS),
                            op=ALU.add,
                        )
                        nc.vector.memset(vaug[:, i, :, HS:HS + 1], 1.0)
                        yield

                def attn_steps(rc):
                    kmax = rc * 4 + 3
                    qt = qt_tiles[rc]
                    for h2 in range(2):
                        psATs = [
                            psapool.tile([HS + 1, 512], dt.float32, name=f"psAT{sub}", tag="psa")
                            for sub in range(2)
                        ]

                        def scores_step(ki):
                            rel = max(0, ki * 128 - rc * 512)
                            ps2 = ps2pool.tile([128, 2, 512], dt.float32, name="ps2", tag="ps2")
                            for sub in range(2):
                                pb = sub * 64
                                nc.tensor.matmul(
                                    ps2[:, sub, rel:512],
                                    kt[h2][pb:pb + 64, ki * 128:(ki + 1) * 128],
                                    qt[pb:pb + 64, h2, rel:512],
                                    start=True, stop=True,
                                )
                            pt2 = ptpool.tile([128, 2, 512], dt.bfloat16, name="pt2", tag="pt2")
                            nc.scalar.activation(pt2[:, :, rel:512], ps2[:, :, rel:512], AF.Exp)
                            if rel > 0:
                                nc.vector.memset(pt2[:, :, 0:rel], 0.0)
                            if ki * 128 - rc * 512 >= 0:
                                for sub in range(2):
                                    nc.vector.tensor_tensor(
                                        pt2[:, sub, rel:rel + 128], pt2[:, sub, rel:rel + 128],
                                        mask_sb[:], op=ALU.mult,
                                    )
                            return pt2

                        pending = scores_step(0)
                        for ki in range(kmax + 1):
                            nxt = scores_step(ki + 1) if ki < kmax else None
                            for sub in range(2):
                                nc.tensor.matmul(
                                    psATs[sub][:],
                                    vaug[:, ki, h2 * 2 + sub, :],
                                    pending[:, sub, :],
                                    start=(ki == 0), stop=(ki == kmax),
                                )
                            pending = nxt
                            yield
                        aT2 = avnpool.tile([128, 512], dt.bfloat16, name="aT2", tag=f"aT2{h2}")
                        for sub in range(2):
                            avt = anpool.tile([HS + 1, 512], dt.bfloat16, name="avt", tag="avt")
                            nc.scalar.copy(avt[:], psATs[sub][:])
                            # 1/den per query, in row layout: exp(-ln(den)) stays
                            # in the natural_log_exp ACT set (no table switch).
                            lnd = dbpool.tile([1, 512], dt.float32, name="lnd", tag="lnd")
                            nc.scalar.activation(lnd[:], avt[HS:HS + 1, :], AF.Ln)
                            rden = dbpool.tile([1, 512], dt.bfloat16, name="rden", tag="rden")
                            nc.scalar.activation(rden[:], lnd[:], AF.Exp, scale=-1.0)
                            # broadcast the recip row across 64 partitions via a
                            # contract-1 matmul, then scale A^T columns on DVE.
                            psB = psb.tile([HS, 512], dt.float32, name="psB", tag="psb")
                            nc.tensor.matmul(psB[:], ones1_sb[:, 0:HS], rden[:],
                                             start=True, stop=True)
                            if sub == 0:
                                nc.vector.tensor_tensor(aT2[0:HS, :], avt[0:HS, :], psB[:], op=ALU.mult)
                            else:
                                # DVE lanes can't shift partitions; normalize in
                                # place then SBUF->SBUF DMA into rows 64:128.
                                avn = anpool.tile([HS, 512], dt.bfloat16, name="avn", tag="avn1")
                                nc.vector.tensor_tensor(avn[:], avt[0:HS, :], psB[:], op=ALU.mult)
                                nc.sync.dma_start(aT2[HS:128, :], avn[:])
                            yield
                        avn_tiles[rc][h2] = aT2

                def outproj_chunk(rc):
                    split = (rc == 3)
                    for tl in range(4):
                        ob = obpool.tile([128, C], dt.bfloat16, name="ob", tag="ob")
                        for nh in range(2):
                            psO = psb.tile([128, 512], dt.float32, name="psO", tag="psb")
                            for ct in range(2):
                                nc.tensor.matmul(
                                    psO[:],
                                    avn_tiles[rc][ct][:, tl * 128:(tl + 1) * 128],
                                    wo_sb[:, ct, nh * 512:(nh + 1) * 512],
                                    start=(ct == 0), stop=(ct == 1),
                                )
                            nc.vector.tensor_tensor(
                                ob[:, nh * 512:(nh + 1) * 512], psO[:],
                                bob_sb[:, nh * 512:(nh + 1) * 512], op=ALU.add,
                            )
                        if split:
                            nc.sync.dma_start(
                                rs1h[tl // 2][(tl % 2) * 128:(tl % 2 + 1) * 128, :], ob[:])
                            if tl % 2 == 1:
                                nc.gpsimd.collective_compute(
                                    "AllReduce", ALU.add, replica_groups=GROUPS,
                                    ins=[rs1h[tl // 2][:].opt()], outs=[ar1h[tl // 2][:].opt()],
                                )
                        else:
                            nc.sync.dma_start(rs1_in[rc][tl * 128:(tl + 1) * 128, :], ob[:])
                    if not split:
                        nc.gpsimd.collective_compute(
                            "AllReduce", ALU.add, replica_groups=GROUPS,
                            ins=[rs1_in[rc][:].opt()], outs=[ar1_out[rc][:].opt()],
                        )

                def ln2_tile(rc, tl, tail=False):
                    at = obpool.tile([128, C], dt.bfloat16, name="at", tag="ob")
                    if rc == 3:
                        src = ar1h[tl // 2][(tl % 2) * 128:(tl % 2 + 1) * 128, :]
                    else:
                        src = ar1_out[rc][tl * 128:(tl + 1) * 128, :]
                    nc.gpsimd.dma_start(at[:], src)
                    xt = lxpool.tile([128, C], dt.float32, name="xt2", tag="xt")
                    nc.sync.dma_start(
                        xt[:], x_b[(rc * 4 + tl) * 128:(rc * 4 + tl + 1) * 128, :]
                    )
                    # residual add on GpSimd (idle engine) so the DVE critical
                    # chain starts at bn_stats; on the tail chunk latency wins
                    # over offload, keep it on DVE.
                    z = zpool.tile([128, C], dt.float32, name="z", tag="z")
                    if tail:
                        nc.vector.tensor_tensor(z[:], at[:], xt[:], op=ALU.add)
                    else:
                        nc.gpsimd.tensor_tensor(z[:], at[:], xt[:], op=ALU.add)
                    h2n = hpool.tile([128, C], dt.bfloat16, name="h2n", tag="h")
                    ln_tile(z[:], h2n[:])
                    for j in range(NCT):
                        transpose_128(h2T[rc][:, j, tl * 128:(tl + 1) * 128],
                                      h2n[:, j * 128:(j + 1) * 128], vec=True)
                    zb = zbpool.tile([128, C], dt.bfloat16, name="zbt", tag="zbt")
                    nc.vector.scalar_tensor_tensor(
                        zb[:], z[:], 1.0 / TP, bq4_sb[:], op0=ALU.mult, op1=ALU.add
                    )
                    zb_tiles[rc][tl] = zb

                def ln2_chunk(rc):
                    for tl in range(4):
                        ln2_tile(rc, tl)

                def mlp_steps(rc):
                    # batch the 8 gelus AFTER all MLP1 matmuls: consecutive
                    # Gelu ops on ACT share one table-set load instead of
                    # thrashing against interleaved attention Exp ops.
                    uts = []
                    for ht in range(HIDR // 128):
                        psU = psb.tile([128, 512], dt.float32, name="psU", tag="psb")
                        for j in range(NCT):
                            nc.tensor.matmul(
                                psU[:],
                                w1_sb[:, j, ht * 128:(ht + 1) * 128],
                                h2T[rc][:, j, :],
                                start=(j == 0), stop=(j == NCT - 1),
                            )
                        ut = utpool.tile([128, 512], dt.bfloat16, name="ut", tag="ut")
                        nc.any.tensor_copy(ut[:], psU[:])
                        uts.append(ut)
                        yield
                    gts = []
                    for ht in range(HIDR // 128):
                        gt = gtpool.tile([128, 512], dt.bfloat16, name="gt", tag="gt")
                        nc.scalar.activation(
                            gt[:], uts[ht][:], AF.Gelu, bias=b1_sb[:, ht:ht + 1]
                        )
                        gts.append(gt)
                    yield
                    for tl in range(4):
                        mb = mbpool.tile([128, C], dt.bfloat16, name="mb", tag="mb")
                        for nh in range(2):
                            psD = psb.tile([128, 512], dt.float32, name="psD", tag="psb")
                            for ht in range(HIDR // 128):
                                nc.tensor.matmul(
                                    psD[:],
                                    gts[ht][:, tl * 128:(tl + 1) * 128],
                                    w2_sb[:, ht, nh * 512:(nh + 1) * 512],
                                    start=(ht == 0), stop=(ht == HIDR // 128 - 1),
                                )
                            nc.vector.tensor_tensor(
                                mb[:, nh * 512:(nh + 1) * 512], psD[:],
                                zb_tiles[rc][tl][:, nh * 512:(nh + 1) * 512],
                                op=ALU.add,
                            )
                        nc.sync.dma_start(rs2_in[rc][tl * 128:(tl + 1) * 128, :], mb[:])
                        yield
                    nc.gpsimd.collective_compute(
                        "ReduceScatter", ALU.add, replica_groups=GROUPS,
                        ins=[rs2_in[rc][:].opt()], outs=[rs2_out[rc][:].opt()],
                    )

                def tail3_steps():
                    """Chunk 3: LN2 tiles -> half-chunk (N=256) MLP1 -> MLP2,
                    pipelined per 256-row half so the tail has no full-chunk
                    barrier."""
                    rc = 3
                    for half in range(2):
                        ln2_tile(rc, 2 * half, tail=True)
                        yield
                        ln2_tile(rc, 2 * half + 1, tail=True)
                        yield
                        uts_h = []
                        for ht in range(HIDR // 128):
                            psU = psb.tile([128, 256], dt.float32, name="psU3", tag="psb")
                            for j in range(NCT):
                                nc.tensor.matmul(
                                    psU[:],
                                    w1_sb[:, j, ht * 128:(ht + 1) * 128],
                                    h2T[rc][:, j, half * 256:(half + 1) * 256],
                                    start=(j == 0), stop=(j == NCT - 1),
                                )
                            ut3 = g3pool.tile([128, 256], dt.bfloat16, name="ut3", tag=f"u3_{ht}")
                            nc.any.tensor_copy(ut3[:], psU[:])
                            uts_h.append(ut3)
                            yield
                        gts_h = []
                        for ht in range(HIDR // 128):
                            gt3 = g3pool.tile([128, 256], dt.bfloat16, name="gt3", tag=f"g3_{ht}")
                            nc.scalar.activation(
                                gt3[:], uts_h[ht][:], AF.Gelu, bias=b1_sb[:, ht:ht + 1]
                            )
                            gts_h.append(gt3)
                        yield
                        for tl in (2 * half, 2 * half + 1):
                            mb = mbpool.tile([128, C], dt.bfloat16, name="mb", tag="mb")
                            for nh in range(2):
                                psD = psb.tile([128, 512], dt.float32, name="psD", tag="psb")
                                for ht in range(HIDR // 128):
                                    nc.tensor.matmul(
                                        psD[:],
                                        gts_h[ht][:, (tl % 2) * 128:(tl % 2 + 1) * 128],
                                        w2_sb[:, ht, nh * 512:(nh + 1) * 512],
                                        start=(ht == 0), stop=(ht == HIDR // 128 - 1),
                                    )
                                nc.vector.tensor_tensor(
                                    mb[:, nh * 512:(nh + 1) * 512], psD[:],
                                    zb_tiles[rc][tl][:, nh * 512:(nh + 1) * 512],
                                    op=ALU.add,
                                )
                            nc.sync.dma_start(rs2_in[rc][tl * 128:(tl + 1) * 128, :], mb[:])
                            yield
                    nc.gpsimd.collective_compute(
                        "ReduceScatter", ALU.add, replica_groups=GROUPS,
                        ins=[rs2_in[rc][:].opt()], outs=[rs2_out[rc][:].opt()],
                    )

                def final_chunk(rc):
                    nc.gpsimd.dma_start(out[rc * 128:(rc + 1) * 128, :], rs2_out[rc][:])

                def drain(gen):
                    for _ in gen:
                        pass

                def interleave(gen_a, gen_b, na, nb):
                    """Merge two instruction generators proportionally."""
                    ia = ib = 0
                    done_a = done_b = False
                    while not (done_a and done_b):
                        pick_a = (not done_a) and (done_b or ia * nb <= ib * na)
                        if pick_a:
                            try:
                                next(gen_a)
                                ia += 1
                            except StopIteration:
                                done_a = True
                        else:
                            try:
                                next(gen_b)
                                ib += 1
                            except StopIteration:
                                done_b = True

                def n_attn(rc):
                    return 2 * (rc * 4 + 4) + 4

                N_LNQKV0 = 12
                N_LNQKV = 9
                N_MLP = 13

                # ---- interleaved chunk-pipelined schedule
                interleave(ln1_own(), lnqkv_steps(0), 3, N_LNQKV0)  # AG1s early
                interleave(attn_steps(0), lnqkv_steps(1), n_attn(0), N_LNQKV)
                outproj_chunk(0)                     # AR1(0)
                interleave(attn_steps(1), lnqkv_steps(2), n_attn(1), N_LNQKV)
                ln2_chunk(0)
                outproj_chunk(1)                     # AR1(1)
                interleave(attn_steps(2), lnqkv_steps(3), n_attn(2), N_LNQKV)
                drain(mlp_steps(0))                  # RS2(0)
                ln2_chunk(1)
                outproj_chunk(2)                     # AR1(2)
                interleave(attn_steps(3), mlp_steps(1), n_attn(3), N_MLP)  # RS2(1)
                final_chunk(0)
                outproj_chunk(3)                     # AR1(3a), AR1(3b) early
                final_chunk(1)
                ln2_chunk(2)
                # chunk-3 tail rides under chunk-2's MLP matmuls
                interleave(tail3_steps(), mlp_steps(2), 26, N_MLP)  # RS2(2), RS2(3)
                final_chunk(2)
                final_chunk(3)

    _split_sync_waits(nc)
    return nc


@functools.lru_cache(maxsize=1)
def _get_nc():
    return _build_nc()


def _make_in_maps(inputs):
    x = np.asarray(inputs["x"], F32)
    W_qkv = np.asarray(inputs["W_qkv"], F32)
    b_qkv = np.asarray(inputs["b_qkv"], F32)
    W_o = np.asarray(inputs["W_o"], F32)
    b_o = np.asarray(inputs["b_o"], F32)
    ln1_g = np.asarray(inputs["ln1_g"], F32)
    ln1_b = np.asarray(inputs["ln1_b"], F32)
    ln2_g = np.asarray(inputs["ln2_g"], F32)
    ln2_b = np.asarray(inputs["ln2_b"], F32)
    W1 = np.asarray(inputs["W1"], F32)
    b1 = np.asarray(inputs["b1"], F32)
    W2 = np.asarray(inputs["W2"], F32)
    b2 = np.asarray(inputs["b2"], F32)

    scale = HS ** -0.5
    Wqkv_f = ln1_g[:, None] * W_qkv
    bqkv_f = ln1_b @ W_qkv + b_qkv
    Kw, Qw, Vw = Wqkv_f[:, :C], Wqkv_f[:, C:2 * C], Wqkv_f[:, 2 * C:]
    bK, bQ, bV = bqkv_f[:C], bqkv_f[C:2 * C], bqkv_f[2 * C:]
    W1f = ln2_g[:, None] * W1
    b1f = ln2_b @ W1 + b1

    ident = np.eye(128, dtype=BF16)
    mask = np.triu(np.ones((128, 128), dtype=F32)).astype(BF16)
    bob4 = np.ascontiguousarray(np.broadcast_to(b_o / TP, (128, C))).astype(F32)
    b2qc = np.ascontiguousarray(np.broadcast_to(b2 / TP, (128, C))).astype(F32)

    in_maps = []
    for core in range(NCORES):
        g, r = divmod(core, TP)
        hs = slice(CHR * r, CHR * (r + 1))
        hid = slice(HIDR * r, HIDR * (r + 1))
        xg = x[g]
        xown = np.concatenate(
            [xg[cc * 512 + r * 128: cc * 512 + (r + 1) * 128] for cc in (1, 2, 3)]
        )
        m = {
            "x_b": np.ascontiguousarray(xg),
            "x_own": np.ascontiguousarray(xown),
            "wq": np.ascontiguousarray(Qw[:, hs] * scale).astype(BF16),
            "wk": np.ascontiguousarray(Kw[:, hs]).astype(BF16),
            "wv": np.ascontiguousarray(Vw[:, hs]).astype(BF16),
            "bq": np.ascontiguousarray((bQ[hs] * scale).reshape(2, 128).T),
            "bk": np.ascontiguousarray(bK[hs].reshape(2, 128).T),
            "bvb": np.ascontiguousarray(np.broadcast_to(bV[hs], (128, CHR))),
            "wo": np.ascontiguousarray(W_o[hs, :]).astype(BF16),
            "bob": bob4,
            "w1": np.ascontiguousarray(W1f[:, hid]).astype(BF16),
            "b1": np.ascontiguousarray(b1f[hid].reshape(HIDR // 128, 128).T),
            "w2": np.ascontiguousarray(W2[hid, :]).astype(BF16),
            "bq4": b2qc,
            "ident": ident,
            "maskut": mask,
        }
        in_maps.append(m)
    return in_maps


def _run(inputs, trace=False):
    nc = _get_nc()
    in_maps = _make_in_maps(inputs)
    res = bass_utils.run_bass_kernel_spmd(
        nc, in_maps, core_ids=list(range(NCORES)), trace=trace
    )
    out = np.empty((B, T, C), F32)
    for core in range(NCORES):
        g, r = divmod(core, TP)
        o = np.asarray(res.results[core]["out"], dtype=F32)
        for rc in range(TP):
            out[g, rc * 512 + r * 128: rc * 512 + (r + 1) * 128] = o[rc * 128:(rc + 1) * 128]
    return out, res


def kernel(**inputs) -> np.ndarray:
    out, _ = _run(inputs, trace=False)
    return out


# revision 30
# speedup vs baseline: 1.1862x; 1.1862x over previous
"""Trainium2 Bass kernel for a dense transformer block (B=2, T=2048, C=1024, 16 heads).

Sharding: data-parallel over batch (2 groups of 4 cores) x tensor-parallel
within each group (4 heads + 1024 MLP hidden per core). The T=2048 rows are
processed as four 512-row chunks through a software-pipelined schedule:

  LN1+QKV(chunk) -> attention(chunk) -> out-proj -> AllReduce(bf16)
    -> residual+LN2 (replicated in group) -> MLP -> ReduceScatter(bf16) -> out

v2 changes vs v1 (trace-driven):
  - LN1 for chunks 1-3 is sequence-parallel: each core normalizes+transposes
    only its own 128 rows per chunk; one merged AllGather distributes hT.
  - LN rstd computed as exp(-0.5*ln(var+eps)) so the whole kernel (attention
    exp included) stays in the natural_log_exp ACT table set: no table thrash.
  - Attention softmax epilogue: instead of transposing A^T to normalize rows,
    the denominator row is partition-broadcast on GpSimd and A^T columns are
    divided in place; out-proj consumes the normalized A^T directly via
    contract-64 matmuls (kills the PE<->DVE transpose ping-pong).
  - Scores for both 64-channel subheads share one [128,2,512] PSUM tile and
    a single Exp instruction.
  - MLP gelu reads PSUM directly with the bias fused (no DVE copy stage).
  - b_o/TP folded into the out-proj PSUM evacuation; chunk-3 MLP row-split
    so the tail pipelines LN2-tile -> MLP1-tile.
"""
import functools
import os
import sys
import types

sys.path.insert(0, "/opt/trn_rl_repo")

import numpy as np
import ml_dtypes

import concourse.bass as bass
import concourse.mybir as mybir
from concourse import tile
import concourse.bass_utils as bass_utils

BF16 = ml_dtypes.bfloat16
F32 = np.float32
dt = mybir.dt
AF = mybir.ActivationFunctionType
ALU = mybir.AluOpType

B, T, C = 2, 2048, 1024
NH, HS = 16, 64
NCORES = 8
TP = 4                      # tensor-parallel group size
GROUPS = [[0, 1, 2, 3], [4, 5, 6, 7]]
HPR = NH // TP              # heads per rank
CHR = HPR * HS              # attn channels per rank (256)
HIDR = 4 * C // TP          # MLP hidden per rank (1024)
RPC = T // TP               # rows per core (512)
EPS = 1e-5
NCT = C // 128              # C tiles (8)
NRT = T // 128              # row tiles over full T (16)


# ---------------------------------------------------------------------------
# Harness fixups: the walrus in this container caps sync-wait commands per
# instruction, but Tile's kernel-tail drain carries one wait per active
# processor. Split those waits onto individual SP nops ahead of the drain.
def _patched_drain_and_barrier(self, tick_clock, wait_clock):
    nc = self.nc
    probe = mybir.InstNoOp(
        name=nc.get_next_instruction_name(),
        engine=mybir.EngineType.SP,
        bass_nofuse=True,
    )
    wait_clock.add_sem_waits(probe, tile.ScopedClock({None: tick_clock.global_clock}))
    waits = list(probe.sync_info.on_wait) if probe.sync_info is not None else []
    for w in waits:
        nop = nc.sync.nop(nofuse=True, hint="split_tail_wait")
        nop.ins.sync_info = mybir.SyncInfo(on_wait=[w], on_update=[])
    nc.sync.drain()
    nc.all_engine_barrier()
    assert self.sems is not None
    popped = nc._tile_sem_poison_stack.pop()
    assert popped is self._sem_poison
    nc.clear_and_free_semaphores(list(self.sems.allocated().values()))
    nc.all_engine_barrier()


tile.TileContext._drain_and_barrier = _patched_drain_and_barrier


def _install_ntff_hook():
    """antenv.axon_hooks is absent from this image; provide it and register
    the ctypes NTFF profile hook so trace=True yields exec_time_ns."""
    if "antenv.axon_hooks" in sys.modules:
        return
    import antenv

    mod = types.ModuleType("antenv.axon_hooks")
    mod._hook = None
    mod.set_axon_ntff_profile_hook = lambda h: setattr(mod, "_hook", h)
    mod.get_axon_ntff_profile_hook = lambda: mod._hook
    sys.modules["antenv.axon_hooks"] = mod
    antenv.axon_hooks = mod
    try:
        from trn_agent_boot.trn_boot import _ntff_profile_via_ctypes

        hook = _ntff_profile_via_ctypes("/opt/axon/libaxon_pjrt.so")
        if hook is not None:
            mod.set_axon_ntff_profile_hook(hook)
    except Exception:
        pass
    bass_utils.upload_artifacts = lambda tmpdir: f"local://{tmpdir}"

    import concourse.bass2jax as b2j

    orig_hook = b2j.neuronx_cc_hook

    def dbg_hook(*a, **k):
        try:
            return orig_hook(*a, **k)
        except BaseException:
            import traceback

            traceback.print_exc()
            raise

    b2j.neuronx_cc_hook = dbg_hook


_install_ntff_hook()


_SYNC_WAIT_LIMIT = 1


def _split_sync_waits(nc, limit=_SYNC_WAIT_LIMIT):
    """Walrus in this container rejects instructions with more than a couple
    of sync-wait commands; hoist excess waits onto same-engine NOPs placed
    immediately before the offending instruction."""
    n_split = 0
    for fn in nc.m.functions:
        for bb in fn.blocks:
            new_insts = []
            for inst in bb.instructions:
                si = inst.sync_info
                if si is not None and si.on_wait is not None and len(si.on_wait) > limit:
                    waits = list(si.on_wait)
                    for idx, w in enumerate(waits[limit:]):
                        nop = mybir.InstNoOp(
                            name=f"{inst.name}-sw{idx}",
                            engine=inst.engine,
                            bass_nofuse=True,
                            sync_info=mybir.SyncInfo(on_wait=[w], on_update=[]),
                        )
                        new_insts.append(nop)
                        n_split += 1
                    inst.sync_info = mybir.SyncInfo(
                        on_wait=waits[:limit], on_update=list(si.on_update)
                    )
                new_insts.append(inst)
            bb.instructions = new_insts
    return n_split


# ---------------------------------------------------------------------------
def _build_nc() -> bass.Bass:
    nc = bass.Bass("TRN2", num_devices=NCORES, num_swdge_queues=4)

    x_b = nc.dram_tensor("x_b", [T, C], dt.float32, kind="ExternalInput")
    x_own = nc.dram_tensor("x_own", [3 * 128, C], dt.float32, kind="ExternalInput")
    wq = nc.dram_tensor("wq", [C, CHR], dt.bfloat16, kind="ExternalInput")
    wk = nc.dram_tensor("wk", [C, CHR], dt.bfloat16, kind="ExternalInput")
    wv = nc.dram_tensor("wv", [C, CHR], dt.bfloat16, kind="ExternalInput")
    bq = nc.dram_tensor("bq", [128, 2], dt.float32, kind="ExternalInput")
    bk = nc.dram_tensor("bk", [128, 2], dt.float32, kind="ExternalInput")
    bvb = nc.dram_tensor("bvb", [128, CHR], dt.float32, kind="ExternalInput")
    wo = nc.dram_tensor("wo", [CHR, C], dt.bfloat16, kind="ExternalInput")
    bob = nc.dram_tensor("bob", [128, C], dt.float32, kind="ExternalInput")
    w1 = nc.dram_tensor("w1", [C, HIDR], dt.bfloat16, kind="ExternalInput")
    b1 = nc.dram_tensor("b1", [128, HIDR // 128], dt.float32, kind="ExternalInput")
    w2 = nc.dram_tensor("w2", [HIDR, C], dt.bfloat16, kind="ExternalInput")
    bq4 = nc.dram_tensor("bq4", [128, C], dt.float32, kind="ExternalInput")
    ident = nc.dram_tensor("ident", [128, 128], dt.bfloat16, kind="ExternalInput")
    maskut = nc.dram_tensor("maskut", [128, 128], dt.bfloat16, kind="ExternalInput")
    out = nc.dram_tensor("out", [RPC, C], dt.bfloat16, kind="ExternalOutput")

    with tile.TileContext(nc) as tc:
        with (
            tc.tile_pool(name="dram", bufs=1, space="DRAM") as dram,
            tc.tile_pool(name="const", bufs=1) as cpool,
            tc.tile_pool(name="kqv", bufs=1) as kqvpool,
        ):
            rs1_in = [dram.tile([512, C], dt.bfloat16, name=f"rs1i{rc}", tag=f"rs1i{rc}") for rc in range(3)]
            ar1_out = [dram.tile([512, C], dt.bfloat16, name=f"ar1o{rc}", tag=f"ar1o{rc}") for rc in range(3)]
            rs1h = [dram.tile([256, C], dt.bfloat16, name=f"rs1h{k}", tag=f"rs1h{k}") for k in range(2)]
            ar1h = [dram.tile([256, C], dt.bfloat16, name=f"ar1h{k}", tag=f"ar1h{k}") for k in range(2)]
            rs2_in = [dram.tile([512, C], dt.bfloat16, name=f"rs2i{rc}", tag=f"rs2i{rc}") for rc in range(TP)]
            rs2_out = [dram.tile([128, C], dt.bfloat16, name=f"rs2o{rc}", tag=f"rs2o{rc}") for rc in range(TP)]
            ag1i = [dram.tile([NCT, 128, 128], dt.bfloat16, name=f"ag1i{k}", tag=f"ag1i{k}") for k in range(3)]
            ag1o = [dram.tile([TP, NCT, 128, 128], dt.bfloat16, name=f"ag1o{k}", tag=f"ag1o{k}") for k in range(3)]
            warm_in = dram.tile([128, 4], dt.float32, name="warm_i", tag="warm_i")
            warm_out = dram.tile([TP * 128, 4], dt.float32, name="warm_o", tag="warm_o")
            nc.gpsimd.collective_compute(
                "AllGather", ALU.bypass, replica_groups=GROUPS,
                ins=[warm_in[:].opt()], outs=[warm_out[:].opt()],
            )

            # ---- weights/constants to SBUF
            wq_sb = cpool.tile([128, NCT, CHR], dt.bfloat16, name="wq", tag="wq")
            wk_sb = cpool.tile([128, NCT, CHR], dt.bfloat16, name="wk", tag="wk")
            wv_sb = cpool.tile([128, NCT, CHR], dt.bfloat16, name="wv", tag="wv")
            nc.scalar.dma_start(wq_sb[:], wq.rearrange("(j p) o -> p j o", p=128))
            nc.scalar.dma_start(wk_sb[:], wk.rearrange("(j p) o -> p j o", p=128))
            nc.scalar.dma_start(wv_sb[:], wv.rearrange("(j p) o -> p j o", p=128))
            wo_sb = cpool.tile([128, 2, C], dt.bfloat16, name="wo", tag="wo")
            nc.scalar.dma_start(wo_sb[:], wo.rearrange("(t p) o -> p t o", p=128))
            w1_sb = cpool.tile([128, NCT, HIDR], dt.bfloat16, name="w1", tag="w1")
            nc.scalar.dma_start(w1_sb[:], w1.rearrange("(j p) o -> p j o", p=128))
            w2_sb = cpool.tile([128, HIDR // 128, C], dt.bfloat16, name="w2", tag="w2")
            nc.scalar.dma_start(w2_sb[:], w2.rearrange("(j p) o -> p j o", p=128))
            bq_sb = cpool.tile([128, 2], dt.float32, name="bq", tag="bq")
            bk_sb = cpool.tile([128, 2], dt.float32, name="bk", tag="bk")
            nc.scalar.dma_start(bq_sb[:], bq[:])
            nc.scalar.dma_start(bk_sb[:], bk[:])
            bvb_sb = cpool.tile([128, CHR], dt.float32, name="bvb", tag="bvb")
            nc.scalar.dma_start(bvb_sb[:], bvb[:])
            bob_sb = cpool.tile([128, C], dt.float32, name="bob", tag="bob")
            nc.scalar.dma_start(bob_sb[:], bob[:])
            b1_sb = cpool.tile([128, HIDR // 128], dt.float32, name="b1", tag="b1")
            nc.scalar.dma_start(b1_sb[:], b1[:])
            bq4_sb = cpool.tile([128, C], dt.float32, name="bq4", tag="bq4")
            nc.scalar.dma_start(bq4_sb[:], bq4[:])
            id_sb = cpool.tile([128, 128], dt.bfloat16, name="id", tag="id")
            nc.sync.dma_start(id_sb[:], ident[:])
            mask_sb = cpool.tile([128, 128], dt.bfloat16, name="mask", tag="mask")
            nc.sync.dma_start(mask_sb[:], maskut[:])
            eps_sb = cpool.tile([128, 1], dt.float32, name="eps", tag="eps")
            nc.vector.memset(eps_sb[:], EPS)
            ones1_sb = cpool.tile([1, 64], dt.bfloat16, name="ones1", tag="ones1")
            nc.vector.memset(ones1_sb[:], 1.0)

            kt = [kqvpool.tile([128, T], dt.bfloat16, name=f"kt{h2}", tag=f"kt{h2}") for h2 in range(2)]
            vaug = kqvpool.tile([128, NRT, HPR, HS + 1], dt.bfloat16, name="vaug", tag="vaug")
            h2T = [kqvpool.tile([128, NCT, 512], dt.bfloat16, name=f"h2T{rc}", tag=f"h2T{rc % 2}") for rc in range(TP)]
            zb_tiles = [[None] * 4 for _ in range(TP)]
            hT_tiles = [None] * TP
            qt_tiles = [None] * TP
            avn_tiles = [[None] * 2 for _ in range(TP)]

            from contextlib import ExitStack
            with ExitStack() as stack:
                ep = stack.enter_context
                lxpool = ep(tc.tile_pool(name="lnx", bufs=2))
                spool = ep(tc.tile_pool(name="lnsp", bufs=6))
                hpool = ep(tc.tile_pool(name="lnh", bufs=3))
                hTpool = ep(tc.tile_pool(name="hTc", bufs=2))
                htopool = ep(tc.tile_pool(name="hTo", bufs=2))
                qtpool = ep(tc.tile_pool(name="qtc", bufs=2))
                zpool = ep(tc.tile_pool(name="zt", bufs=2))
                zbpool = ep(tc.tile_pool(name="zb", bufs=6))
                ptpool = ep(tc.tile_pool(name="pt", bufs=3))
                anpool = ep(tc.tile_pool(name="anat", bufs=2))
                dbpool = ep(tc.tile_pool(name="dnb", bufs=2))
                avnpool = ep(tc.tile_pool(name="avn", bufs=2))
                obpool = ep(tc.tile_pool(name="ob", bufs=2))
                gtpool = ep(tc.tile_pool(name="gt", bufs=8))
                utpool = ep(tc.tile_pool(name="ut", bufs=8))
                g3pool = ep(tc.tile_pool(name="g3", bufs=1))
                mbpool = ep(tc.tile_pool(name="mb", bufs=2))
                psb = ep(tc.tile_pool(name="psb", bufs=2, space="PSUM"))
                ps2pool = ep(tc.tile_pool(name="ps2", bufs=2, space="PSUM"))
                psapool = ep(tc.tile_pool(name="psa", bufs=2, space="PSUM"))
                # LN of a [128, C] f32 tile -> bf16 (gain/bias folded downstream)
                # rstd = exp(-0.5*ln(var+eps)): keeps ACT in natural_log_exp set.
                def ln_tile(src_ap, dst_ap):
                    st6 = spool.tile([128, 2, 6], dt.float32, name="st6", tag="st6")
                    nc.vector.bn_stats(st6[:, 0, :], src_ap[:, 0:512])
                    nc.vector.bn_stats(st6[:, 1, :], src_ap[:, 512:1024])
                    st2 = spool.tile([128, 2], dt.float32, name="st2", tag="st2")
                    nc.vector.bn_aggr(st2[:], st6[:])
                    lnv = spool.tile([128, 1], dt.float32, name="lnv", tag="lnv")
                    nc.scalar.activation(lnv[:], st2[:, 1:2], AF.Ln, bias=eps_sb[:])
                    rstd = spool.tile([128, 1], dt.float32, name="rstd", tag="rstd")
                    nc.scalar.activation(rstd[:], lnv[:], AF.Exp, scale=-0.5)
                    nc.vector.tensor_scalar(
                        dst_ap, src_ap, st2[:, 0:1], rstd[:],
                        op0=ALU.subtract, op1=ALU.mult,
                    )

                def transpose_128(dst_ap, src_ap, vec=False):
                    pst = psb.tile([128, 128], dt.bfloat16, name="pst", tag="psb")
                    nc.tensor.transpose(pst[:], src_ap, id_sb[:])
                    if vec:
                        nc.vector.tensor_copy(dst_ap, pst[:])
                    else:
                        nc.scalar.copy(dst_ap, pst[:])

                def ln1_own():
                    """Sequence-parallel LN1 for chunks 1-3 (own 128 rows each);
                    per-chunk AllGather of the transposed slices so chunk 1's
                    hT is available as early as possible."""
                    for idx in range(3):
                        xt = lxpool.tile([128, C], dt.float32, name="xt", tag="xt")
                        nc.sync.dma_start(xt[:], x_own[idx * 128:(idx + 1) * 128, :])
                        ho = hpool.tile([128, C], dt.bfloat16, name="h", tag="h")
                        ln_tile(xt[:], ho[:])
                        hTo = htopool.tile([128, NCT, 128], dt.bfloat16, name="hTo", tag="hTo")
                        for j in range(NCT):
                            transpose_128(hTo[:, j, :], ho[:, j * 128:(j + 1) * 128])
                        nc.sync.dma_start(ag1i[idx][:].rearrange("j p c -> p j c"), hTo[:])
                        nc.gpsimd.collective_compute(
                            "AllGather", ALU.bypass, replica_groups=GROUPS,
                            ins=[ag1i[idx][:].opt()], outs=[ag1o[idx][:].opt()],
                        )
                        yield

                def lnqkv_steps(cc):
                    """hT for chunk cc (local LN for cc=0, AG DMA-in for cc>0),
                    then K/Q/V projections for the chunk (generator)."""
                    hT = hTpool.tile([128, NCT, 512], dt.bfloat16, name="hTc", tag="hTc")
                    hT_tiles[cc] = hT
                    if cc == 0:
                        for tl in range(4):
                            xt = lxpool.tile([128, C], dt.float32, name="xt", tag="xt")
                            nc.sync.dma_start(xt[:], x_b[tl * 128:(tl + 1) * 128, :])
                            h = hpool.tile([128, C], dt.bfloat16, name="h", tag="h")
                            ln_tile(xt[:], h[:])
                            for j in range(NCT):
                                transpose_128(hT[:, j, tl * 128:(tl + 1) * 128],
                                              h[:, j * 128:(j + 1) * 128])
                            yield
                    else:
                        for r in range(TP):
                            nc.sync.dma_start(
                                hT[:, :, r * 128:(r + 1) * 128],
                                ag1o[cc - 1][r].rearrange("j p c -> p j c"),
                            )
                        yield
                    qt = qtpool.tile([128, 2, 512], dt.bfloat16, name="qtc", tag="qtc")
                    qt_tiles[cc] = qt
                    for h2 in range(2):
                        for w_sb, is_q, b_sb in ((wk_sb, False, bk_sb), (wq_sb, True, bq_sb)):
                            ps = psb.tile([128, 512], dt.float32, name="psqk", tag="psb")
                            for j in range(NCT):
                                nc.tensor.matmul(
                                    ps[:],
                                    w_sb[:, j, h2 * 128:(h2 + 1) * 128],
                                    hT[:, j, :],
                                    start=(j == 0), stop=(j == NCT - 1),
                                )
                            if is_q:
                                nc.scalar.activation(
                                    qt[:, h2, :], ps[:],
                                    AF.Identity, bias=b_sb[:, h2:h2 + 1],
                                )
                            else:
                                nc.scalar.activation(
                                    kt[h2][:, cc * 512:(cc + 1) * 512], ps[:],
                                    AF.Identity, bias=b_sb[:, h2:h2 + 1],
                                )
                            yield
                    for tl in range(4):
                        i = cc * 4 + tl
                        ps = psb.tile([128, CHR], dt.float32, name="psv", tag="psb")
                        for j in range(NCT):
                            nc.tensor.matmul(
                                ps[:],
                                hT[:, j, tl * 128:(tl + 1) * 128],
                                wv_sb[:, j, :],
                                start=(j == 0), stop=(j == NCT - 1),
                            )
                        nc.vector.tensor_tensor(
                            vaug[:, i, :, 0:HS],
                            ps[:].rearrange("p (h d) -> p h d", d=HS),
                            bvb_sb[:].rearrange("p (h d) -> p h d", d=HS),
                            op=ALU.add,
                        )
                        nc.vector.memset(vaug[:, i, :, HS:HS + 1], 1.0)
                        yield

                def attn_steps(rc):
                    kmax = rc * 4 + 3
                    qt = qt_tiles[rc]
                    for h2 in range(2):
                        psATs = [
                            psapool.tile([HS + 1, 512], dt.float32, name=f"psAT{sub}", tag="psa")
                            for sub in range(2)
                        ]

                        def scores_step(ki):
                            rel = max(0, ki * 128 - rc * 512)
                            ps2 = ps2pool.tile([128, 2, 512], dt.float32, name="ps2", tag="ps2")
                            for sub in range(2):
                                pb = sub * 64
                                nc.tensor.matmul(
                                    ps2[:, sub, rel:512],
                                    kt[h2][pb:pb + 64, ki * 128:(ki + 1) * 128],
                                    qt[pb:pb + 64, h2, rel:512],
                                    start=True, stop=True,
                                )
                            pt2 = ptpool.tile([128, 2, 512], dt.bfloat16, name="pt2", tag="pt2")
                            nc.scalar.activation(pt2[:, :, rel:512], ps2[:, :, rel:512], AF.Exp)
                            if rel > 0:
                                nc.vector.memset(pt2[:, :, 0:rel], 0.0)
                            if ki * 128 - rc * 512 >= 0:
                                for sub in range(2):
                                    nc.vector.tensor_tensor(
                                        pt2[:, sub, rel:rel + 128], pt2[:, sub, rel:rel + 128],
                                        mask_sb[:], op=ALU.mult,
                                    )
                            return pt2

                        pending = scores_step(0)
                        for ki in range(kmax + 1):
                            nxt = scores_step(ki + 1) if ki < kmax else None
                            for sub in range(2):
                                nc.tensor.matmul(
                                    psATs[sub][:],
                                    vaug[:, ki, h2 * 2 + sub, :],
                                    pending[:, sub, :],
                                    start=(ki == 0), stop=(ki == kmax),
                                )
                            pending = nxt
                            yield
                        aT2 = avnpool.tile([128, 512], dt.bfloat16, name="aT2", tag=f"aT2{h2}")
                        for sub in range(2):
                            avt = anpool.tile([HS + 1, 512], dt.bfloat16, name="avt", tag="avt")
                            nc.scalar.copy(avt[:], psATs[sub][:])
                            # 1/den per query, in row layout: exp(-ln(den)) stays
                            # in the natural_log_exp ACT set (no table switch).
                            lnd = dbpool.tile([1, 512], dt.float32, name="lnd", tag="lnd")
                            nc.scalar.activation(lnd[:], avt[HS:HS + 1, :], AF.Ln)
                            rden = dbpool.tile([1, 512], dt.bfloat16, name="rden", tag="rden")
                            nc.scalar.activation(rden[:], lnd[:], AF.Exp, scale=-1.0)
                            # broadcast the recip row across 64 partitions via a
                            # contract-1 matmul, then scale A^T columns on DVE.
                            psB = psb.tile([HS, 512], dt.float32, name="psB", tag="psb")
                            nc.tensor.matmul(psB[:], ones1_sb[:, 0:HS], rden[:],
                                             start=True, stop=True)
                            if sub == 0:
                                nc.vector.tensor_tensor(aT2[0:HS, :], avt[0:HS, :], psB[:], op=ALU.mult)
                            else:
                                # DVE lanes can't shift partitions; normalize in
                                # place then SBUF->SBUF DMA into rows 64:128.
                                avn = anpool.tile([HS, 512], dt.bfloat16, name="avn", tag="avn1")
                                nc.vector.tensor_tensor(avn[:], avt[0:HS, :], psB[:], op=ALU.mult)
                                nc.sync.dma_start(aT2[HS:128, :], avn[:])
                            yield
                        avn_tiles[rc][h2] = aT2

                def outproj_chunk(rc):
                    split = (rc == 3)
                    for tl in range(4):
                        ob = obpool.tile([128, C], dt.bfloat16, name="ob", tag="ob")
                        for nh in range(2):
                            psO = psb.tile([128, 512], dt.float32, name="psO", tag="psb")
                            for ct in range(2):
                                nc.tensor.matmul(
                                    psO[:],
                                    avn_tiles[rc][ct][:, tl * 128:(tl + 1) * 128],
                                    wo_sb[:, ct, nh * 512:(nh + 1) * 512],
                                    start=(ct == 0), stop=(ct == 1),
                                )
                            nc.vector.tensor_tensor(
                                ob[:, nh * 512:(nh + 1) * 512], psO[:],
                                bob_sb[:, nh * 512:(nh + 1) * 512], op=ALU.add,
                            )
                        if split:
                            nc.sync.dma_start(
                                rs1h[tl // 2][(tl % 2) * 128:(tl % 2 + 1) * 128, :], ob[:])
                            if tl % 2 == 1:
                                nc.gpsimd.collective_compute(
                                    "AllReduce", ALU.add, replica_groups=GROUPS,
                                    ins=[rs1h[tl // 2][:].opt()], outs=[ar1h[tl // 2][:].opt()],
                                )
                        else:
                            nc.sync.dma_start(rs1_in[rc][tl * 128:(tl + 1) * 128, :], ob[:])
                    if not split:
                        nc.gpsimd.collective_compute(
                            "AllReduce", ALU.add, replica_groups=GROUPS,
                            ins=[rs1_in[rc][:].opt()], outs=[ar1_out[rc][:].opt()],
                        )

                def ln2_tile(rc, tl, tail=False):
                    at = obpool.tile([128, C], dt.bfloat16, name="at", tag="ob")
                    if rc == 3:
                        src = ar1h[tl // 2][(tl % 2) * 128:(tl % 2 + 1) * 128, :]
                    else:
                        src = ar1_out[rc][tl * 128:(tl + 1) * 128, :]
                    nc.gpsimd.dma_start(at[:], src)
                    xt = lxpool.tile([128, C], dt.float32, name="xt2", tag="xt")
                    nc.sync.dma_start(
                        xt[:], x_b[(rc * 4 + tl) * 128:(rc * 4 + tl + 1) * 128, :]
                    )
                    # residual add on GpSimd (idle engine) so the DVE critical
                    # chain starts at bn_stats; on the tail chunk latency wins
                    # over offload, keep it on DVE.
                    z = zpool.tile([128, C], dt.float32, name="z", tag="z")
                    if tail:
                        nc.vector.tensor_tensor(z[:], at[:], xt[:], op=ALU.add)
                    else:
                        nc.gpsimd.tensor_tensor(z[:], at[:], xt[:], op=ALU.add)
                    h2n = hpool.tile([128, C], dt.bfloat16, name="h2n", tag="h")
                    ln_tile(z[:], h2n[:])
                    for j in range(NCT):
                        transpose_128(h2T[rc][:, j, tl * 128:(tl + 1) * 128],
                                      h2n[:, j * 128:(j + 1) * 128], vec=True)
                    zb = zbpool.tile([128, C], dt.bfloat16, name="zbt", tag="zbt")
                    nc.vector.scalar_tensor_tensor(
                        zb[:], z[:], 1.0 / TP, bq4_sb[:], op0=ALU.mult, op1=ALU.add
                    )
                    zb_tiles[rc][tl] = zb

                def ln2_chunk(rc):
                    for tl in range(4):
                        ln2_tile(rc, tl)

                def mlp_steps(rc):
                    # batch the 8 gelus AFTER all MLP1 matmuls: consecutive
                    # Gelu ops on ACT share one table-set load instead of
                    # thrashing against interleaved attention Exp ops.
                    uts = []
                    for ht in range(HIDR // 128):
                        psU = psb.tile([128, 512], dt.float32, name="psU", tag="psb")
                        for j in range(NCT):
                            nc.tensor.matmul(
                                psU[:],
                                w1_sb[:, j, ht * 128:(ht + 1) * 128],
                                h2T[rc][:, j, :],
                                start=(j == 0), stop=(j == NCT - 1),
                            )
                        ut = utpool.tile([128, 512], dt.bfloat16, name="ut", tag="ut")
                        nc.any.tensor_copy(ut[:], psU[:])
                        uts.append(ut)
                        yield
                    gts = []
                    for ht in range(HIDR // 128):
                        gt = gtpool.tile([128, 512], dt.bfloat16, name="gt", tag="gt")
                        nc.scalar.activation(
                            gt[:], uts[ht][:], AF.Gelu, bias=b1_sb[:, ht:ht + 1]
                        )
                        gts.append(gt)
                    yield
                    for tl in range(4):
                        mb = mbpool.tile([128, C], dt.bfloat16, name="mb", tag="mb")
                        for nh in range(2):
                            psD = psb.tile([128, 512], dt.float32, name="psD", tag="psb")
                            for ht in range(HIDR // 128):
                                nc.tensor.matmul(
                                    psD[:],
                                    gts[ht][:, tl * 128:(tl + 1) * 128],
                                    w2_sb[:, ht, nh * 512:(nh + 1) * 512],
                                    start=(ht == 0), stop=(ht == HIDR // 128 - 1),
                                )
                            nc.vector.tensor_tensor(
                                mb[:, nh * 512:(nh + 1) * 512], psD[:],
                                zb_tiles[rc][tl][:, nh * 512:(nh + 1) * 512],
                                op=ALU.add,
                            )
                        nc.sync.dma_start(rs2_in[rc][tl * 128:(tl + 1) * 128, :], mb[:])
                        yield
                    nc.gpsimd.collective_compute(
                        "ReduceScatter", ALU.add, replica_groups=GROUPS,
                        ins=[rs2_in[rc][:].opt()], outs=[rs2_out[rc][:].opt()],
                    )

                def tail3_steps():
                    """Chunk 3: LN2 tiles -> half-chunk (N=256) MLP1 -> MLP2,
                    pipelined per 256-row half so the tail has no full-chunk
                    barrier."""
                    rc = 3
                    for half in range(2):
                        ln2_tile(rc, 2 * half, tail=True)
                        yield
                        ln2_tile(rc, 2 * half + 1, tail=True)
                        yield
                        uts_h = []
                        for ht in range(HIDR // 128):
                            psU = psb.tile([128, 256], dt.float32, name="psU3", tag="psb")
                            for j in range(NCT):
                                nc.tensor.matmul(
                                    psU[:],
                                    w1_sb[:, j, ht * 128:(ht + 1) * 128],
                                    h2T[rc][:, j, half * 256:(half + 1) * 256],
                                    start=(j == 0), stop=(j == NCT - 1),
                                )
                            ut3 = g3pool.tile([128, 256], dt.bfloat16, name="ut3", tag=f"u3_{ht}")
                            nc.any.tensor_copy(ut3[:], psU[:])
                            uts_h.append(ut3)
                            yield
                        gts_h = []
                        for ht in range(HIDR // 128):
                            gt3 = g3pool.tile([128, 256], dt.bfloat16, name="gt3", tag=f"g3_{ht}")
                            nc.scalar.activation(
                                gt3[:], uts_h[ht][:], AF.Gelu, bias=b1_sb[:, ht:ht + 1]
                            )
                            gts_h.append(gt3)
                        yield
                        for tl in (2 * half, 2 * half + 1):
                            mb = mbpool.tile([128, C], dt.bfloat16, name="mb", tag="mb")
                            for nh in range(2):
                                psD = psb.tile([128, 512], dt.float32, name="psD", tag="psb")
                                for ht in range(HIDR // 128):
                                    nc.tensor.matmul(
                                        psD[:],
                                        gts_h[ht][:, (tl % 2) * 128:(tl % 2 + 1) * 128],
                                        w2_sb[:, ht, nh * 512:(nh + 1) * 512],
                                        start=(ht == 0), stop=(ht == HIDR // 128 - 1),
                                    )
                                nc.vector.tensor_tensor(
                                    mb[:, nh * 512:(nh + 1) * 512], psD[:],
                                    zb_tiles[rc][tl][:, nh * 512:(nh + 1) * 512],
                                    op=ALU.add,
                                )
                            nc.sync.dma_start(rs2_in[rc][tl * 128:(tl + 1) * 128, :], mb[:])
                            yield
                    nc.gpsimd.collective_compute(
                        "ReduceScatter", ALU.add, replica_groups=GROUPS,
                        ins=[rs2_in[rc][:].opt()], outs=[rs2_out[rc][:].opt()],
                    )

                def final_chunk(rc):
                    nc.gpsimd.dma_start(out[rc * 128:(rc + 1) * 128, :], rs2_out[rc][:])

                def drain(gen):
                    for _ in gen:
                        pass

                def interleave(gen_a, gen_b, na, nb):
                    """Merge two instruction generators proportionally."""
                    ia = ib = 0
                    done_a = done_b = False
                    while not (done_a and done_b):
                        pick_a = (not done_a) and (done_b or ia * nb <= ib * na)
                        if pick_a:
                            try:
                                next(gen_a)
                                ia += 1
                            except StopIteration:
                                done_a = True
                        else:
                            try:
                                next(gen_b)
                                ib += 1
                            except StopIteration:
                                done_b = True

                def n_attn(rc):
                    return 2 * (rc * 4 + 4) + 4

                N_LNQKV0 = 12
                N_LNQKV = 9
                N_MLP = 13

                # ---- interleaved chunk-pipelined schedule
                interleave(ln1_own(), lnqkv_steps(0), 3, N_LNQKV0)  # AG1s early
                interleave(attn_steps(0), lnqkv_steps(1), n_attn(0), N_LNQKV)
                outproj_chunk(0)                     # AR1(0)
                interleave(attn_steps(1), lnqkv_steps(2), n_attn(1), N_LNQKV)
                ln2_chunk(0)
                outproj_chunk(1)                     # AR1(1)
                interleave(attn_steps(2), lnqkv_steps(3), n_attn(2), N_LNQKV)
                drain(mlp_steps(0))                  # RS2(0)
                ln2_chunk(1)
                outproj_chunk(2)                     # AR1(2)
                interleave(attn_steps(3), mlp_steps(1), n_attn(3), N_MLP)  # RS2(1)
                final_chunk(0)
                outproj_chunk(3)                     # AR1(3a), AR1(3b) early
                final_chunk(1)
                ln2_chunk(2)
                # chunk-3 tail rides under chunk-2's MLP matmuls
                interleave(tail3_steps(), mlp_steps(2), 26, N_MLP)  # RS2(2), RS2(3)
                final_chunk(2)
                final_chunk(3)

    _split_sync_waits(nc)
    return nc


@functools.lru_cache(maxsize=1)
def _get_nc():
    return _build_nc()


def _make_in_maps(inputs):
    x = np.asarray(inputs["x"], F32)
    W_qkv = np.asarray(inputs["W_qkv"], F32)
    b_qkv = np.asarray(inputs["b_qkv"], F32)
    W_o = np.asarray(inputs["W_o"], F32)
    b_o = np.asarray(inputs["b_o"], F32)
    ln1_g = np.asarray(inputs["ln1_g"], F32)
    ln1_b = np.asarray(inputs["ln1_b"], F32)
    ln2_g = np.asarray(inputs["ln2_g"], F32)
    ln2_b = np.asarray(inputs["ln2_b"], F32)
    W1 = np.asarray(inputs["W1"], F32)
    b1 = np.asarray(inputs["b1"], F32)
    W2 = np.asarray(inputs["W2"], F32)
    b2 = np.asarray(inputs["b2"], F32)

    scale = HS ** -0.5
    Wqkv_f = ln1_g[:, None] * W_qkv
    bqkv_f = ln1_b @ W_qkv + b_qkv
    Kw, Qw, Vw = Wqkv_f[:, :C], Wqkv_f[:, C:2 * C], Wqkv_f[:, 2 * C:]
    bK, bQ, bV = bqkv_f[:C], bqkv_f[C:2 * C], bqkv_f[2 * C:]
    W1f = ln2_g[:, None] * W1
    b1f = ln2_b @ W1 + b1

    ident = np.eye(128, dtype=BF16)
    mask = np.triu(np.ones((128, 128), dtype=F32)).astype(BF16)
    bob4 = np.ascontiguousarray(np.broadcast_to(b_o / TP, (128, C))).astype(F32)
    b2qc = np.ascontiguousarray(np.broadcast_to(b2 / TP, (128, C))).astype(F32)

    in_maps = []
    for core in range(NCORES):
        g, r = divmod(core, TP)
        hs = slice(CHR * r, CHR * (r + 1))
        hid = slice(HIDR * r, HIDR * (r + 1))
        xg = x[g]
        xown = np.concatenate(
            [xg[cc * 512 + r * 128: cc * 512 + (r + 1) * 128] for cc in (1, 2, 3)]
        )
        m = {
            "x_b": np.ascontiguousarray(xg),
            "x_own": np.ascontiguousarray(xown),
            "wq": np.ascontiguousarray(Qw[:, hs] * scale).astype(BF16),
            "wk": np.ascontiguousarray(Kw[:, hs]).astype(BF16),
            "wv": np.ascontiguousarray(Vw[:, hs]).astype(BF16),
            "bq": np.ascontiguousarray((bQ[hs] * scale).reshape(2, 128).T),
            "bk": np.ascontiguousarray(bK[hs].reshape(2, 128).T),
            "bvb": np.ascontiguousarray(np.broadcast_to(bV[hs], (128, CHR))),
            "wo": np.ascontiguousarray(W_o[hs, :]).astype(BF16),
            "bob": bob4,
            "w1": np.ascontiguousarray(W1f[:, hid]).astype(BF16),
            "b1": np.ascontiguousarray(b1f[hid].reshape(HIDR // 128, 128).T),
            "w2": np.ascontiguousarray(W2[hid, :]).astype(BF16),
            "bq4": b2qc,
            "ident": ident,
            "maskut": mask,
        }
        in_maps.append(m)
    return in_maps


def _run(inputs, trace=False):
    nc = _get_nc()
    in_maps = _make_in_maps(inputs)
    res = bass_utils.run_bass_kernel_spmd(
        nc, in_maps, core_ids=list(range(NCORES)), trace=trace
    )
    out = np.empty((B, T, C), F32)
    for core in range(NCORES):
        g, r = divmod(core, TP)
        o = np.asarray(res.results[core]["out"], dtype=F32)
        for rc in range(TP):
            out[g, rc * 512 + r * 128: rc * 512 + (r + 1) * 128] = o[rc * 128:(rc + 1) * 128]
    return out, res


def kernel(**inputs) -> np.ndarray:
    out, _ = _run(inputs, trace=False)
    return out
